# revision 33
# baseline (speedup 1.0000x reference)
"""GCN-Attention kernel for Trainium2, data-parallel over 8 NeuronCores.

Reference computation (per image b of 64, category c of 100):
  full = concat(image_features, bbox)                    [N, 2052]
  x[b,c,:] = sum_{boxes n in bucket(b,c), slot<3} lin_w[slot]*full[n] + lin_b
  support  = x @ gc_w                                    [B, 100, 2048]
  gcn      = leaky_relu((X + adj) @ support + gc_b)
  out[b]   = global_features[b] @ gcn[b]                 [B, 2048]

Host prep (pure input reorganization, <0.3% of total FLOPs): the occurrence-
slot scatter is resolved into the weighted sum x on the host.

Algebraic restructure: the bbox columns (4) and the lin_b bias do NOT get
their own phase-2 contraction chunk.  Because
  A_b @ (x_bbox_b @ W_bbox) = (A_b @ x_bbox_b) @ W_bbox          (rank 4)
  A_b @ (lin_b * ones ⊗ colsum(W)) = lin_b * rowsum(A_b) ⊗ colsum(W)
both fold into phase 3 as 5 extra contraction rows, costing zero extra
matmuls.  Phase 2 contracts exactly K=2048 = 16 full 128-chunks.

Row packing: the 800 (image,category) rows pack into 7 partition tiles
[100,100,120,120,120,120,120] (matmul cost is set by the moving free dim
only, so fewer output row-tiles = fewer matmuls: 7x16x4=448 vs per-image
512; boundaries at 100/200 are image-aligned so only 4 images straddle
tiles -> 12 phase-3 pieces).  Each tile keeps spare partitions holding the
shared extras rows [lin_b*colsum(W); W_bbox], so phase 3 streams a tile's
rows 0..kk as moving operand (base partition 0 — HW requires partition-0-
aligned matmul operands) against a per-(image,tile) stationary block that
carries the adjacency weights for that image's categories in that tile
(zeros elsewhere, extras weights on the image's first tile only).

Phases per 512-col chunk n (4 chunks):
  phase 2: tile groups {0..3} then {4..6}; k-major interleaved chains for
           the two n=0 units (DMA-paced), tile-major after (16 back-to-back
           accumulating matmuls per bank), casting each tile as it
           completes and enqueueing finished images' phase-3/4.
  phase 3: per image, 1-2 accumulating matmuls (K<=126) + scalar Lrelu.
  phase 4: attention row matmul [K=100 -> 1, 512], DVE copy, 2KB DMA out.
  phase-3/4 items pop between phase-2 matmuls, slot-gated so a p4 never
  pops before its p3's Lrelu has drained (an unready matmul stalls the
  in-order PE queue).

DMA: the tile framework rotates 8 completion semaphores per ring and a
consumer waits on the latest same-slot count at its emission time, so any
ring with more than 8 input DMAs creates false dependencies where early
consumers wait on late transfers.  Hence: at most 8 input DMAs per ring
(big consolidated pieces), ordered by consumption time.  sync (SP has no
other work — ring-full blocking is harmless) and gpsimd (software DGE)
carry the bulk; scalar issues only early pieces (before its first cast)
plus the 2KB output DMAs.
"""
import os
import time

import ml_dtypes
import numpy as np

import concourse.bacc as bacc
import concourse.mybir as mybir
import concourse.tile as tile
from concourse import bass_utils

B = 64
C = 100
LOOP = 3
FEAT = 2052
OUT = 2048
NCORES = 8
BPC = B // NCORES  # images per core
ROWS = BPC * C     # (image,category) rows per core
NKC = 16           # K chunks of 128 (image-feature contraction only)
NCH = 4            # 512-col output chunks
NT = 7             # packed row tiles
TSZ = [100, 100, 120, 120, 120, 120, 120]
TOFF = [0, 100, 200, 320, 440, 560, 680]
EXR = [100, 100, 120, 120, 120, 120, 120]  # extras row offset per tile

f32 = mybir.dt.float32
bf16 = mybir.dt.bfloat16
np_bf16 = ml_dtypes.bfloat16

_programs: dict = {}
last_results = None  # BassKernelResults of the most recent run (for harnesses)


def _pieces():
    """Per image: list of (tile, global piece index, row range in tile).

    Piece j's stationary block lives at columns [100j, 100j+100) of adjB.
    """
    out = []
    j = 0
    for b in range(BPC):
        g0, g1 = b * C, (b + 1) * C
        ps = []
        for t in range(NT):
            lo, hi = max(g0, TOFF[t]), min(g1, TOFF[t] + TSZ[t])
            if lo < hi:
                ps.append((t, j, lo - TOFF[t], hi - TOFF[t]))
                j += 1
        out.append(ps)
    return out


PIECES = _pieces()
NPIECE = sum(len(p) for p in PIECES)  # 12


def _occ_slots(key):
    """Occurrence index among equal-valued keys, stable order (matches jax ref)."""
    n = key.shape[0]
    order = np.argsort(key, kind="stable")
    sk = key[order]
    idx = np.arange(n)
    is_new = np.concatenate([[True], sk[1:] != sk[:-1]]) if n else np.zeros(0, bool)
    run_start = np.maximum.accumulate(np.where(is_new, idx, 0))
    pos = idx - run_start
    slots = np.zeros(n, np.int64)
    slots[order] = pos
    return slots


def _build(has_gcb: bool):
    nc = bacc.Bacc("TRN2", target_bir_lowering=False, debug=False,
                   num_devices=NCORES)

    KN = 6 if has_gcb else 5   # extras rows per tile

    # gc_w in quad-pair form: pair p of chunk n holds K-chunks 8p..8p+7 as
    # two 2048-col quads side by side (8KB contiguous per-partition lines)
    gcw2_d = nc.dram_tensor("gcw2", [NCH, 2, 128, 2 * OUT], bf16,
                            kind="ExternalInput").ap()
    # x^T with all 16 K-chunks concatenated along the free dim
    xtp_d = nc.dram_tensor("xtp", [128, NKC * ROWS], bf16,
                           kind="ExternalInput").ap()
    adjB_d = nc.dram_tensor("adjB", [127, NPIECE * C], bf16,
                            kind="ExternalInput").ap()
    # extras replicated per tile: [KN, NT*OUT]
    extr_d = nc.dram_tensor("extr", [KN, NT * OUT], bf16,
                            kind="ExternalInput").ap()
    gT_d = nc.dram_tensor("gT", [C, BPC], bf16, kind="ExternalInput").ap()
    out_d = nc.dram_tensor("out", [BPC, OUT], f32, kind="ExternalOutput").ap()

    T0 = [0, 1, 2, 3]
    T1 = [4, 5, 6]
    B0 = [b for b in range(BPC) if all(t <= 3 for t, _, _, _ in PIECES[b])]
    B1 = [b for b in range(BPC) if b not in B0]

    with tile.TileContext(nc) as tc:
        with tc.tile_pool(name="const", bufs=1) as cpool, \
             tc.tile_pool(name="sb", bufs=1) as pool, \
             tc.tile_pool(name="ps", bufs=1, space="PSUM") as psp:

            # ---- SBUF tiles ----
            xtp_sb = cpool.tile([128, NKC * ROWS], bf16, tag="xtp",
                                name="xtp_sb")
            gcw2_sb = [[cpool.tile([128, 2 * OUT], bf16, tag=f"gw{n}{p}",
                                   name=f"gw_sb{n}{p}") for p in range(2)]
                       for n in range(NCH)]
            adjB_sb = cpool.tile([127, NPIECE * C], bf16, tag="adjB",
                                 name="adjB_sb")
            gT_sb = cpool.tile([C, BPC], bf16, tag="gT", name="gT_sb")
            # one packed support tile: tile t at columns [t*OUT,(t+1)*OUT);
            # rows 0..sz-1 phase-2 cast, rows EXR..EXR+KN-1 shared extras
            ssb = pool.tile([126, NT * OUT], bf16, tag="ssb", bufs=1,
                            name="ssb")
            gsbs = [pool.tile([C, OUT], bf16, tag="gsb", bufs=BPC,
                              name=f"gsb_{b}") for b in range(BPC)]

            # ---- DMA delivery: <=8 input DMAs per ring, need-ordered ----
            S, A, G = nc.sync, nc.scalar, nc.gpsimd

            def xt_piece(e, k0, k1, p0=0, p1=128):
                e.dma_start(xtp_sb[p0:p1, k0 * ROWS:k1 * ROWS],
                            xtp_d[p0:p1, k0 * ROWS:k1 * ROWS])

            def gw_piece(e, n, p, c0, c1):
                e.dma_start(gcw2_sb[n][p][:, c0:c1], gcw2_d[n, p, :, c0:c1])

            # sync: k0 half, k1-2, k5-7, k11-13, n1p0, n2p0, n3p0, n3p1
            # scalar: k0 half, k3-4, k8-10, adjB, extras x2, k14-15 [+outs]
            # gpsimd: n0p0 in 1024/1024/2048, n0p1 halves, gT, n1p1, n2p1
            xt_piece(S, 0, 1, 0, 64)
            xt_piece(A, 0, 1, 64, 128)
            gw_piece(G, 0, 0, 0, 1024)
            gw_piece(G, 0, 0, 1024, 2048)
            xt_piece(S, 1, 3)
            xt_piece(A, 3, 5)
            gw_piece(G, 0, 0, 2048, 4096)
            S.dma_start(adjB_sb[:], adjB_d[:])
            A.dma_start(ssb[100:100 + KN, 0:2 * OUT],
                        extr_d[:, 0:2 * OUT])
            A.dma_start(ssb[120:120 + KN, 2 * OUT:NT * OUT],
                        extr_d[:, 2 * OUT:NT * OUT])
            G.dma_start(gT_sb[:], gT_d[:])
            xt_piece(S, 5, 8)
            xt_piece(A, 8, 11)
            gw_piece(G, 0, 1, 0, 2048)
            gw_piece(G, 0, 1, 2048, 4096)
            xt_piece(S, 11, 13)
            xt_piece(A, 13, 16)
            S.dma_start(gcw2_sb[1][0][:], gcw2_d[1, 0])
            G.dma_start(gcw2_sb[1][1][:], gcw2_d[1, 1])
            S.dma_start(gcw2_sb[2][0][:], gcw2_d[2, 0])
            G.dma_start(gcw2_sb[2][1][:], gcw2_d[2, 1])
            S.dma_start(gcw2_sb[3][0][:], gcw2_d[3, 0])
            S.dma_start(gcw2_sb[3][1][:], gcw2_d[3, 1])

            def stat_slice(k, t):
                off = k * ROWS + TOFF[t]
                return xtp_sb[0:128, off:off + TSZ[t]]

            def mov_slice(k, n):
                gt = gcw2_sb[n][k // 8]
                off = ((k % 8) // 4) * OUT + (k % 4) * 512
                return gt[0:128, off:off + 512]

            def cast(i, t, n, src):
                # PSUM -> SBUF bf16 drain, spread across two engines
                dst = ssb[0:TSZ[t], t * OUT + n * 512:t * OUT + (n + 1) * 512]
                if i % 2 == 0:
                    nc.vector.tensor_copy(dst, src)
                else:
                    nc.scalar.activation(dst, src,
                                         mybir.ActivationFunctionType.Copy)

            def walk(g, n, pop_item):
                # k-major (DMA-friendly: chunk k is consumed ~864ns after
                # chunk k-1, matching progressive arrival): interleaved PSUM
                # chains, same-bank revisits 3-4 issues apart
                chains = [psp.tile([128, 512], f32, tag="ch", bufs=4,
                                   name=f"ch_{t}_{n}") for t in g]
                for k in range(NKC):
                    for i, t in enumerate(g):
                        nc.tensor.matmul(
                            chains[i][0:TSZ[t], 0:512],
                            stat_slice(k, t),
                            mov_slice(k, n),
                            start=(k == 0), stop=(k == NKC - 1),
                        )
                    if k >= 3:
                        pop_item()
                for i, t in enumerate(g):
                    cast(i, t, n, chains[i][0:TSZ[t], 0:512])

            def walk_tmajor(g, n, pop_item, enq):
                # steady-state units (all data resident): tile-major — 16
                # back-to-back accumulating matmuls on one bank, casting
                # each tile as it completes and enqueueing the phase-3/4 of
                # images it finishes, so items pipeline under the remaining
                # tiles' matmuls instead of bunching at unit boundaries
                chains = [psp.tile([128, 512], f32, tag="ch", bufs=4,
                                   name=f"chT_{t}_{n}") for t in g]
                for i, t in enumerate(g):
                    for k in range(NKC):
                        nc.tensor.matmul(
                            chains[i][0:TSZ[t], 0:512],
                            stat_slice(k, t),
                            mov_slice(k, n),
                            start=(k == 0), stop=(k == NKC - 1),
                        )
                        if k % 3 == 2:
                            pop_item()
                    cast(i, t, n, chains[i][0:TSZ[t], 0:512])
                    enq(t, n)

            def p3_item(b, n):
                # G[b][:,n] = Lrelu(sum over pieces of block contraction)
                gp = psp.tile([128, 512], f32, tag="gp", bufs=4,
                              name=f"gp_{b}_{n}")
                ps = PIECES[b]
                for idx, (t, j, _, _) in enumerate(ps):
                    kk = EXR[t] + KN
                    nc.tensor.matmul(
                        gp[0:C, 0:512],
                        adjB_sb[0:kk, j * C:(j + 1) * C],
                        ssb[0:kk, t * OUT + n * 512:t * OUT + n * 512 + 512],
                        start=(idx == 0), stop=(idx == len(ps) - 1),
                    )
                nc.scalar.activation(
                    gsbs[b][0:C, n * 512:(n + 1) * 512],
                    gp[0:C, 0:512],
                    mybir.ActivationFunctionType.Lrelu, alpha=0.01,
                )

            def p4_item(b, n):
                op = psp.tile([128, 512], f32, tag="gp", bufs=4,
                              name=f"op_{b}_{n}")
                nc.tensor.matmul(op[0:1, 0:512],
                                 gT_sb[0:C, b:b + 1],
                                 gsbs[b][0:C, n * 512:(n + 1) * 512],
                                 start=True, stop=True)
                ost = pool.tile([1, 512], f32, tag="ostage", bufs=4,
                                name=f"ost_{b}_{n}")
                nc.vector.tensor_copy(ost[0:1, 0:512], op[0:1, 0:512])
                nc.scalar.dma_start(out_d[b:b + 1, n * 512:(n + 1) * 512],
                                    ost[0:1, 0:512])

            queue = []
            slot = [0]

            def pop_item():
                # items gate on a minimum slot so a p4 never pops before its
                # p3's Lrelu (~1.3us) has drained — an unready p4 matmul
                # stalls the whole in-order PE queue
                slot[0] += 1
                if queue and queue[0][0] <= slot[0]:
                    _, kind, b, n = queue.pop(0)
                    if kind == 3:
                        p3_item(b, n)
                        queue.append((slot[0] + 2, 4, b, n))
                    else:
                        p4_item(b, n)

            # image is ready once its last tile is cast
            last_tile = {b: max(t for t, _, _, _ in PIECES[b])
                         for b in range(BPC)}

            def enq(t, n):
                for b in range(BPC):
                    if last_tile[b] == t:
                        queue.append((slot[0] + 1, 3, b, n))

            units = [(g, n) for n in range(NCH) for g in (T0, T1)]
            for u, (g, n) in enumerate(units):
                if u == 1:
                    queue.extend((0, 3, b, 0) for b in B0)
                elif u == 2:
                    queue.extend((0, 3, b, 0) for b in B1)
                if u >= 2:
                    walk_tmajor(g, n, pop_item, enq)
                else:
                    walk(g, n, pop_item)
            while queue:
                pop_item()

    nc.compile()
    return nc


def _get_program(has_gcb: bool = False):
    key = ("pack12", has_gcb)
    if key not in _programs:
        _programs[key] = _build(has_gcb)
    return _programs[key]


def kernel(**inputs) -> np.ndarray:
    global last_results

    imf = np.asarray(inputs["image_features"], np.float32)
    bbox = np.asarray(inputs["bbox_list"], np.float32)
    gf = np.asarray(inputs["global_features"], np.float32)
    adj = np.asarray(inputs["adj"], np.float32)
    X = np.asarray(inputs["X"], np.float32)
    lin_w = np.asarray(inputs["lin_w"], np.float32)
    lin_b = np.float32(np.asarray(inputs["lin_b"]))
    gc_w = np.ascontiguousarray(np.asarray(inputs["gc_w"], np.float32))
    gc_b = np.asarray(inputs["gc_b"], np.float32)
    label = np.asarray(inputs["label_list"]).astype(np.int64)
    batch = np.asarray(inputs["batch"]).astype(np.int64)

    full = np.concatenate([imf, bbox], axis=1)

    # scatter bookkeeping, matching jax semantics: slots by stable order of
    # key=batch*C+(label-1); negative cats wrap, slot>=LOOP / far-oob dropped
    cat = label - 1
    key = batch * C + cat
    slots = _occ_slots(key)
    valid = (slots < LOOP) & (cat >= -C) & (cat < C)
    wvals = np.where(valid, lin_w[np.clip(slots, 0, LOOP - 1)], 0.0).astype(np.float32)
    cidx = np.mod(cat, C).astype(np.int64)

    # host scatter-sum (0.04% of total FLOPs): S[b,c,:] = sum of
    # lin_w[slot]*full over the <=LOOP boxes of bucket (b,c); slots are
    # unique per bucket so per-slot fancy-index adds have no collisions
    S = np.zeros((B, C, FEAT), np.float32)
    bok = valid & (batch >= -B) & (batch < B)
    bmod = np.mod(batch, B)
    for s in range(LOOP):
        sel = bok & (slots == s)
        if np.any(sel):
            S[bmod[sel], cidx[sel]] += wvals[sel, None] * full[sel]

    newadj = X[None, :, :] + adj                               # [B, C, C]
    has_gcb = bool(np.any(gc_b))
    KN = 6 if has_gcb else 5

    # gc_w quad pairs: pair p of chunk n = K-chunks 8p..8p+7, two 2048-col
    # quads side by side
    gcwn = (gc_w[0:2048].reshape(4, 4, 128, NCH, 512).transpose(3, 0, 2, 1, 4)
            .reshape(NCH, 4, 128, OUT))
    gcw2 = np.ascontiguousarray(
        gcwn.reshape(NCH, 2, 2, 128, OUT).transpose(0, 1, 3, 2, 4)
        .reshape(NCH, 2, 128, 2 * OUT)).astype(np_bf16)
    # shared phase-3 extras rows: lin_b*colsum(W_full), W_bbox[, gc_b]
    extr = np.empty((KN, OUT), np.float32)
    extr[0] = lin_b * gc_w.sum(axis=0)
    extr[1:5] = gc_w[2048:FEAT]
    if has_gcb:
        extr[5] = gc_b
    extr7 = np.tile(extr, (1, NT)).astype(np_bf16)

    in_maps = []
    for core in range(NCORES):
        imgs = slice(core * BPC, (core + 1) * BPC)
        Xc = S[imgs].reshape(ROWS, FEAT)
        XT = Xc[:, 0:2048].T                                   # [2048, 800]
        # [128, k*800+row]: per-partition lines hold all 16 K-chunks
        xtp = np.ascontiguousarray(
            XT.reshape(NKC, 128, ROWS).transpose(1, 0, 2)
            .reshape(128, NKC * ROWS)).astype(np_bf16)
        # phase-3 stationary blocks, one [127, 100] column block per
        # (image, tile) piece: adjacency weights for the image's categories
        # at their in-tile row positions; extras weights (rowsum(A),
        # (A@x_bbox)^T[, ones]) at rows EXR[t].. on the image's first piece
        Ac = newadj[imgs]                                      # [8, 100, 100]
        Sbb = S[imgs, :, 2048:FEAT]                            # [8, 100, 4]
        adjB = np.zeros((127, NPIECE * C), np.float32)
        for b in range(BPC):
            A_b = Ac[b]
            for idx, (t, j, r0, r1) in enumerate(PIECES[b]):
                cols = slice(j * C, (j + 1) * C)
                c0 = TOFF[t] + r0 - b * C
                # stat[r, i] = A_b[i, cat(r)]
                adjB[r0:r1, cols] = A_b[:, c0:c0 + (r1 - r0)].T
                if idx == 0:
                    ex = EXR[t]
                    adjB[ex, cols] = A_b.sum(axis=1)
                    adjB[ex + 1:ex + 5, cols] = (A_b @ Sbb[b]).T
                    if has_gcb:
                        adjB[ex + 5, cols] = 1.0
        im = dict(
            gcw2=gcw2,
            xtp=xtp,
            adjB=adjB.astype(np_bf16),
            extr=extr7,
            gT=np.ascontiguousarray(gf[imgs].T).astype(np_bf16),
        )
        in_maps.append(im)

    nc = _get_program(has_gcb)
    res = None
    for attempt in range(4):
        try:
            res = bass_utils.run_bass_kernel_spmd(
                nc, in_maps, core_ids=list(range(NCORES)))
            break
        except Exception:
            if attempt == 3:
                raise
            time.sleep(3 * (attempt + 1))  # transient NRT exec-unit errors
    last_results = res
    return np.concatenate([res.results[i]["out"] for i in range(NCORES)], axis=0)
